# revision 12
# baseline (speedup 1.0000x reference)
"""AMNet GNN forward pass on 8 Trainium2 NeuronCores (Bass/Tile SPMD).

Strategy:
  - Graph preprocessing on host: build the normalized-Laplacian propagation
    matrix A (dense bf16, diagonal 1/lambda_max folded in), sharded by
    destination rows across 8 cores, stored transposed ([src, dst_local])
    so it can be used directly as the matmul stationary operand.
  - Node dim padded 10000 -> 10240 (1280 per core) so everything tiles by 128.
  - MLP replicated on every core (cheap, avoids an AllGather of h).
  - 5 Bernstein propagations: z_{i+1} = A @ z_i as dense TensorE matmuls in
    fp8 e4m3 DoubleRow mode (A host-scaled by 32, descale fused into the
    psum casts; z kept bf16-accurate for attention via a separate staging
    path), z chunks stationary / A streaming, chunked AllGather of the fp8
    z shard (partition-major layout) overlapped with the other half's
    compute.
  - Attention: hfp = tanh(Bxcat @ W_big) with W_big = host-precomputed
    [alpha[f,i] * Wf] stack (folds the Bernstein combination into the GEMM),
    softmax over 5 filters, res = sum_i beta_i(n) * Bx_i.
  - Final: out = tanh(res @ res.T) as a row-sharded GEMM with res^T
    all-gathered; tanh on ScalarE; bf16 output, upcast to f32 on host.
"""

import os
import math
import time

import numpy as np
import ml_dtypes

import concourse.bass as bass
import concourse.mybir as mybir
import concourse.tile as tile
from concourse import bacc
from concourse.bass_utils import run_bass_kernel_spmd
from concourse.masks import make_identity

BF16 = mybir.dt.bfloat16
F8 = mybir.dt.float8e4
ASCALE = 32.0
WSCALE = 16.0
F32 = mybir.dt.float32
NPBF16 = ml_dtypes.bfloat16
NPF8 = ml_dtypes.float8_e4m3

N = 10000          # nodes
IN = 128           # input features
HID = 256          # hidden
FNUM = 5           # filters
KDEG = 5           # Bernstein degree
NB = KDEG + 1      # basis count
NCORES = 8
SH = 1250          # real nodes per core
SHP = 1280         # padded nodes per core
NP_ = NCORES * SHP # padded global nodes (10240)
MCH = SHP // 128   # m-chunks per core (10)
KCH = NP_ // 128   # k-chunks global (80)
SHARED_SPACE = os.environ.get("KERNEL_SHARED", "Shared")
AF = mybir.ActivationFunctionType
ALU = mybir.AluOpType


def _bern_coeff(deg):
    C = np.zeros((deg + 1, deg + 1), dtype=np.float64)
    for k in range(deg + 1):
        for i in range(k, deg + 1):
            C[k, i] = (
                math.comb(deg, k) * math.comb(deg - k, i - k) * ((-1.0) ** (i - k))
            )
    return C


def _pad_id(g):
    return (g // SH) * SHP + (g % SH)


def build_nc(phases=None, skip_bias=False):
    if phases is None:
        phases = int(os.environ.get("KERNEL_PHASES", "4") or "4")
    sub = os.environ.get("KERNEL_SUB", "z")
    nc = bacc.Bacc("TRN2", target_bir_lowering=False, debug=False,
                   num_devices=NCORES)

    amat = nc.declare_dram_parameter("amat", [NP_, SHP], F8, isOutput=False)
    xt = nc.declare_dram_parameter("xt", [IN, NP_], BF16, isOutput=False)
    xtown = nc.declare_dram_parameter("xtown", [IN, SHP], BF16, isOutput=False)
    w1_d = nc.declare_dram_parameter("w1", [IN, HID], BF16, isOutput=False)
    w2_d = nc.declare_dram_parameter("w2", [128, 2, HID], BF16, isOutput=False)
    wx_d = nc.declare_dram_parameter("wx", [128, 2, HID], BF16, isOutput=False)
    wbig_d = nc.declare_dram_parameter("wbig", [128, 12, FNUM * HID], F8,
                                       isOutput=False)
    b1_d = nc.declare_dram_parameter("b1c", [128, 2], F32, isOutput=False)
    b2_d = nc.declare_dram_parameter("b2r", [1, HID], BF16, isOutput=False)
    bx_d = nc.declare_dram_parameter("bxr", [1, HID], BF16, isOutput=False)
    bf_d = nc.declare_dram_parameter("bfc", [1, FNUM * HID], BF16, isOutput=False)
    mfi_d = nc.declare_dram_parameter("mfi", [FNUM, NB], F32, isOutput=False)
    out_d = nc.declare_dram_parameter("out", [SH, N], BF16, isOutput=True)

    with tile.TileContext(nc) as tc:
        with tc.tile_pool(name="const", bufs=1) as cpool, \
             tc.tile_pool(name="persist", bufs=1) as per, \
             tc.tile_pool(name="dram", bufs=1, space="DRAM") as dpool:
            # ---- constants into SBUF ----
            w1_sb = cpool.tile([IN, HID], BF16)
            nc.sync.dma_start(w1_sb[:], w1_d[:])
            w2_sb = cpool.tile([128, 2, HID], BF16)
            nc.sync.dma_start(w2_sb[:], w2_d[:])
            wx_sb = cpool.tile([128, 2, HID], BF16)
            nc.sync.dma_start(wx_sb[:], wx_d[:])
            wbig_sb = cpool.tile([128, 12, FNUM * HID], F8)
            nc.sync.dma_start(wbig_sb[:], wbig_d[:])
            b1_sb = cpool.tile([128, 2], F32)
            nc.sync.dma_start(b1_sb[:], b1_d[:])
            b2_sb = cpool.tile([1, HID], BF16)
            nc.sync.dma_start(b2_sb[:], b2_d[:])
            bx_sb = cpool.tile([1, HID], BF16)
            nc.sync.dma_start(bx_sb[:], bx_d[:])
            bf_sb = cpool.tile([1, FNUM * HID], BF16)
            nc.sync.dma_start(bf_sb[:], bf_d[:])
            mfi_sb = cpool.tile([FNUM, NB], F32)
            nc.sync.dma_start(mfi_sb[:], mfi_d[:])
            ones_sb = cpool.tile([1, 128], BF16)
            nc.vector.memset(ones_sb[:], 1.0)
            idb = cpool.tile([128, 128], BF16)
            make_identity(nc, idb[:])
            idf = cpool.tile([128, 128], F32)
            make_identity(nc, idf[:])

            # ---- persistent SBUF state ----
            z_sb = per.tile([128, NCORES, MCH, HID], F8)      # full z (fp8)
            zof_sb = per.tile([128, MCH, HID], F8)            # own z fp8 stage
            zown_sb = per.tile([128, NB, MCH, HID], BF16)     # own Bx_i shards
            bxT_sb = per.tile([128, 2 * NB, SHP], BF16)       # own Bx_i^T
            bxT8_sb = per.tile([128, 2 * NB, SHP], F8)        # fp8 copy
            resT_sb = per.tile([128, 2, SHP], BF16)           # own res^T
            xp_sb = per.tile([128, MCH, HID], BF16)           # tanh(h@Wx+bx)

            # ---- DRAM bounce buffers for collectives ----
            zbounces = [
                [dpool.tile([128, (6, 4)[g], HID], F8, name=f"zbounce{i}_{g}")
                 for g in range(2)]
                for i in range(KDEG - 1)
            ]
            zgaths = [
                [dpool.tile([NCORES, 128, (6, 4)[g], HID], F8,
                            addr_space=SHARED_SPACE, name=f"zgath{i}_{g}")
                 for g in range(2)]
                for i in range(KDEG - 1)
            ]
            rbounces = [dpool.tile([128, 2, 640], BF16, name=f"rbounce{h}")
                        for h in range(2)]
            rgaths = [dpool.tile([NCORES, 128, 2, 640], BF16,
                                 addr_space=SHARED_SPACE, name=f"rgath{h}")
                      for h in range(2)]

            # tiny warmup collective: pays the cold-start cost of the
            # collectives firmware under the MLP instead of in prop 0
            wub = dpool.tile([1, HID], BF16, name="wub")
            wug = dpool.tile([NCORES, HID], BF16, addr_space=SHARED_SPACE,
                             name="wug")
            nc.vector.memset(ones_sb[:], 1.0)
            nc.sync.dma_start(wub[:], b2_d[:])
            nc.gpsimd.collective_compute(
                "AllGather", ALU.bypass,
                replica_groups=[list(range(NCORES))],
                ins=[wub[:]], outs=[wug[:]])

            def bias_mm(ps_ap, brow_ap):
                # rank-1 matmul adding a free-dim bias row to every partition;
                # elided when the host saw all-zero biases
                if skip_bias:
                    return
                nc.tensor.matmul(ps_ap, ones_sb[0:1, :], brow_ap,
                                 start=False, stop=True)

            # =================== Phase 1: MLP ===================
            # t^T = relu(W1^T x^T + b1): full graph, replicated
            with tc.tile_pool(name="mlp_sb", bufs=1) as mpool, \
                 tc.tile_pool(name="mlp_xt", bufs=3) as xpool, \
                 tc.tile_pool(name="mlp_ps", bufs=4, space="PSUM") as mps:
                tT_sb = mpool.tile([128, 2, NP_], BF16)
                tTo_sb = mpool.tile([128, 2, SHP], BF16)
                for nt in range(NP_ // 512 if sub >= "a" else 0):
                    xt_t = xpool.tile([IN, 512], BF16, tag="xt")
                    nc.sync.dma_start(xt_t[:], xt[:, nt * 512:(nt + 1) * 512])
                    for m in range(2):
                        ps = mps.tile([128, 512], F32, tag="t", bufs=3)
                        nc.tensor.matmul(ps[:], w1_sb[:, m * 128:(m + 1) * 128],
                                         xt_t[:], start=True, stop=True)
                        if (nt + m) % 2 == 0:
                            nc.scalar.activation(
                                tT_sb[:, m, nt * 512:(nt + 1) * 512], ps[:],
                                AF.Relu, bias=b1_sb[:, m:m + 1])
                        else:
                            nc.vector.tensor_scalar(
                                out=tT_sb[:, m, nt * 512:(nt + 1) * 512],
                                in0=ps[:], scalar1=b1_sb[:, m:m + 1],
                                scalar2=0.0, op0=ALU.add, op1=ALU.max)
                # own-shard t^T (same math on own columns; gives rank's shard
                # without rank-dependent addressing)
                off = 0
                for w in ((512, 512, 256) if sub >= "b" else ()):
                    xt_t = xpool.tile([IN, 512], BF16, tag="xt")
                    nc.sync.dma_start(xt_t[:, :w], xtown[:, off:off + w])
                    for m in range(2):
                        ps = mps.tile([128, 512], F32, tag="t", bufs=3)
                        nc.tensor.matmul(ps[:, :w],
                                         w1_sb[:, m * 128:(m + 1) * 128],
                                         xt_t[:, :w], start=True, stop=True)
                        nc.scalar.activation(
                            tTo_sb[:, m, off:off + w], ps[:, :w],
                            AF.Relu, bias=b1_sb[:, m:m + 1])
                    off += w

                # h = t @ W2 + b2 (node-major), full graph -> z_sb
                for nch in range(KCH if sub >= "c" else 0):
                    ps = mps.tile([128, HID], F32, tag="h", bufs=3)
                    for j in range(2):
                        nc.tensor.matmul(
                            ps[:], tT_sb[:, j, nch * 128:(nch + 1) * 128],
                            w2_sb[:, j, :], start=(j == 0), stop=False)
                    bias_mm(ps[:], b2_sb[0:1, :])
                    if nch % 2 == 0:
                        nc.vector.tensor_copy(z_sb[:, nch // MCH, nch % MCH, :],
                                              ps[:])
                    else:
                        nc.scalar.copy(z_sb[:, nch // MCH, nch % MCH, :], ps[:])
                # own shard h -> zown[0] (+ transposes -> bxT[0])
                for mch in range(MCH if sub >= "d" else 0):
                    ps = mps.tile([128, HID], F32, tag="h", bufs=3)
                    for j in range(2):
                        nc.tensor.matmul(
                            ps[:], tTo_sb[:, j, mch * 128:(mch + 1) * 128],
                            w2_sb[:, j, :], start=(j == 0), stop=False)
                    bias_mm(ps[:], b2_sb[0:1, :])
                    nc.vector.tensor_copy(zown_sb[:, 0, mch, :], ps[:])
                    for j in range(2):
                        tp = mps.tile([128, 128], BF16, tag="tp", bufs=2)
                        nc.tensor.transpose(
                            tp[:], zown_sb[:, 0, mch, j * 128:(j + 1) * 128],
                            idb[:])
                        nc.scalar.copy(
                            bxT_sb[:, j, mch * 128:(mch + 1) * 128], tp[:])
                        nc.vector.tensor_copy(
                            bxT8_sb[:, j, mch * 128:(mch + 1) * 128], tp[:])

            if phases < 4:
                dummy = cpool.tile([128, 512], BF16, name="dummy")
                nc.vector.memset(dummy[:], 0.0)
                nc.sync.dma_start(out_d[0:128, 0:512], dummy[:])

            # =================== Phase 2: propagations ===================
            if phases >= 2:
                with tc.tile_pool(name="prop_ps", bufs=6, space="PSUM") as pps, \
                     tc.tile_pool(name="tp_ps", bufs=2, space="PSUM") as tps, \
                     tc.tile_pool(name="aslab", bufs=12) as apool:
                    # xp = tanh(h @ Wx + bx) -- depends only on bxT[0]; emitted
                    # early so it fills PE/ACT idle time during AG waits.
                    for mch in range(MCH):
                        ps = pps.tile([128, HID], F32, tag="acc")
                        for j in range(2):
                            nc.tensor.matmul(
                                ps[:], bxT_sb[:, j, mch * 128:(mch + 1) * 128],
                                wx_sb[:, j, :], start=(j == 0), stop=False)
                        bias_mm(ps[:], bx_sb[0:1, :])
                        nc.scalar.activation(xp_sb[:, mch, :], ps[:], AF.Tanh)

                    # Operand-swapped props: z chunks are the STATIONARY
                    # operand (one LDWEIGHTS per (k, hid-half), reused over
                    # m-tiles), A columns stream as the moving operand.  The
                    # psum output arrives hid-major and IS BxT; the
                    # node-major copy comes from the 20 PE transposes we paid
                    # anyway.  Two passes per prop over own node cols
                    # [0:512] (chunks 0-3) and [512:1280] (chunks 4-9); each
                    # pass's AllGather half ships while the other computes.
                    # k order: chunks 0-3 of every rank first, then 4-9, so
                    # the previous prop's first gathered half unblocks early.
                    korder = [r * (MCH // 2) + pc
                              for half in ((2, 3, 4), (0, 1))
                              for pc in half for r in range(NCORES)]
                    NPAIR = KCH // 2
                    PASSES = [(512, (512, 256), 4, MCH), (0, (512,), 0, 4)]
                    for i in range(KDEG):
                        war_guard = None
                        for g, (base, tiles, clo, chi) in enumerate(PASSES):
                            wcols = sum(tiles)
                            accs = {}
                            for j in range(2):
                                for ti in range(len(tiles)):
                                    accs[j, ti] = pps.tile(
                                        [128, 512], F32, tag="acc",
                                        name=f"acc_{i}_{g}_{j}_{ti}")
                            for ki, kp in enumerate(korder):
                                r, pc = kp // (MCH // 2), kp % (MCH // 2)
                                sl = apool.tile([128, 2, 768], F8, tag="a")
                                nc.sync.dma_start(
                                    sl[:, :, :wcols],
                                    amat[kp * 256:(kp + 1) * 256,
                                         base:base + wcols].rearrange(
                                             "(i p) m -> p i m", p=128))
                                for j in range(2):
                                    lhs = z_sb[:, r, 2 * pc:2 * pc + 2,
                                               j * 128:(j + 1) * 128]
                                    off = 0
                                    for ti, tw in enumerate(tiles):
                                        mm = nc.tensor.matmul(
                                            accs[j, ti][:, :tw], lhs,
                                            sl[:, :, off:off + tw],
                                            start=(ki == 0),
                                            stop=(ki == NPAIR - 1),
                                            perf_mode=(
                                                mybir.MatmulPerfMode.DoubleRow))
                                        off += tw
                                        if (g == 1 and ki == 3 * NCORES - 1
                                                and j == 1
                                                and ti == len(tiles) - 1):
                                            # last matmul of prop i reading
                                            # z_sb chunks 4-9 (PE in-order)
                                            war_guard = mm
                            # psums are z_{i+1}^T pieces -> straight to bxT
                            for j in range(2):
                                off = 0
                                for ti, tw in enumerate(tiles):
                                    dst = bxT_sb[:, (i + 1) * 2 + j,
                                                 base + off:base + off + tw]
                                    dst8 = bxT8_sb[:, (i + 1) * 2 + j,
                                                   base + off:base + off + tw]
                                    if (j + ti) % 2 == 0:
                                        nc.vector.tensor_scalar_mul(
                                            dst, accs[j, ti][:, :tw],
                                            1.0 / ASCALE)
                                        nc.scalar.activation(
                                            dst8, accs[j, ti][:, :tw], AF.Copy,
                                            scale=1.0 / ASCALE)
                                    else:
                                        nc.scalar.activation(
                                            dst, accs[j, ti][:, :tw], AF.Copy,
                                            scale=1.0 / ASCALE)
                                        nc.vector.tensor_scalar_mul(
                                            dst8, accs[j, ti][:, :tw],
                                            1.0 / ASCALE)
                                    off += tw
                            # node-major zown via PE transpose of bxT blocks
                            for mch in range(clo, chi):
                                for j in range(2):
                                    tp = tps.tile([128, 128], BF16, tag="tp")
                                    nc.tensor.transpose(
                                        tp[:],
                                        bxT_sb[:, (i + 1) * 2 + j,
                                               mch * 128:(mch + 1) * 128],
                                        idb[:])
                                    nc.scalar.copy(
                                        zown_sb[:, i + 1, mch,
                                                j * 128:(j + 1) * 128], tp[:])
                                    if i < KDEG - 1:
                                        nc.vector.tensor_copy(
                                            zof_sb[:, mch,
                                                   j * 128:(j + 1) * 128],
                                            tp[:])
                            # chunked AllGather of this pass's node chunks
                            if i < KDEG - 1:
                                nc.sync.dma_start(
                                    zbounces[i][g][:],
                                    zof_sb[:, clo:chi, :])
                                nc.gpsimd.collective_compute(
                                    "AllGather", ALU.bypass,
                                    replica_groups=[list(range(NCORES))],
                                    ins=[zbounces[i][g][:]],
                                    outs=[zgaths[i][g][:]])
                        # land the gathered halves into z_sb for prop i+1.
                        # The chunks 0-3 write-after-read hazard against this
                        # prop's late reads is not auto-tracked; pin it.
                        if i < KDEG - 1:
                            for g, (_, _, clo, chi) in enumerate(PASSES):
                                for r in range(NCORES):
                                    d = nc.sync.dma_start(
                                        z_sb[:, r, clo:chi, :],
                                        zgaths[i][g][r])
                                    if g == 0 and war_guard is not None:
                                        tile.add_dep_helper(
                                            d.ins, war_guard.ins, sync=True,
                                            reason="z_sb WAR vs prop reads")

            # =================== Phase 3: attention ===================
            if phases >= 3:
                with tc.tile_pool(name="att_ps", bufs=4, space="PSUM") as aps, \
                     tc.tile_pool(name="att_ps2", bufs=1, space="PSUM") as aps2, \
                     tc.tile_pool(name="att_sb", bufs=2) as asb, \
                     tc.tile_pool(name="att_small", bufs=4) as asm:
                    nsl = [(0, 512), (512, 512), (1024, FNUM * HID - 1024)]
                    for mch in range(MCH):
                        hfp = asb.tile([128, FNUM * HID], BF16, tag="hfp")
                        for (noff, nw) in nsl:
                            ps = aps.tile([128, 512], F32, tag="hf")
                            for pc in range(NB):
                                nc.tensor.matmul(
                                    ps[:, :nw],
                                    bxT8_sb[:, 2 * pc:2 * pc + 2,
                                            mch * 128:(mch + 1) * 128],
                                    wbig_sb[:, 2 * pc:2 * pc + 2,
                                            noff:noff + nw],
                                    start=(pc == 0),
                                    stop=(pc == NB - 1 and skip_bias),
                                    perf_mode=mybir.MatmulPerfMode.DoubleRow)
                            nc.tensor.matmul(ps[:, :nw], ones_sb[0:1, :],
                                             bf_sb[0:1, noff:noff + nw],
                                             start=False, stop=True) \
                                if not skip_bias else None
                            nc.scalar.activation(hfp[:, noff:noff + nw],
                                                 ps[:, :nw], AF.Tanh,
                                                 scale=1.0 / WSCALE)
                        # logits over filters
                        logit = asm.tile([128, FNUM], F32, tag="logit")
                        scr = asb.tile([128, HID], BF16, tag="scr")
                        for f in range(FNUM):
                            nc.vector.tensor_mul(
                                scr[:], hfp[:, f * HID:(f + 1) * HID],
                                xp_sb[:, mch, :])
                            nc.vector.reduce_sum(
                                out=logit[:, f:f + 1], in_=scr[:],
                                axis=mybir.AxisListType.X)
                        # softmax over the FNUM free dim
                        mx = asm.tile([128, 1], F32, tag="mx")
                        nc.vector.reduce_max(out=mx[:], in_=logit[:],
                                             axis=mybir.AxisListType.X)
                        score = asm.tile([128, FNUM], F32, tag="score")
                        nc.vector.tensor_scalar(out=score[:], in0=logit[:],
                                                scalar1=mx[:, 0:1], scalar2=None,
                                                op0=ALU.subtract)
                        nc.scalar.activation(score[:], score[:], AF.Exp)
                        sm = asm.tile([128, 1], F32, tag="sm")
                        nc.vector.reduce_sum(out=sm[:], in_=score[:],
                                             axis=mybir.AxisListType.X)
                        rs = asm.tile([128, 1], F32, tag="rs")
                        nc.vector.reciprocal(rs[:], sm[:])
                        nc.vector.tensor_scalar_mul(score[:], score[:], rs[:, 0:1])
                        # beta = score @ (gate*alpha):  transpose score, small GEMM
                        tp = aps2.tile([FNUM, 128], F32, tag="scT")
                        nc.tensor.transpose(tp[:], score[:], idf[:])
                        scT = asm.tile([FNUM, 128], F32, tag="scTs")
                        nc.vector.tensor_copy(scT[:], tp[:])
                        bps = aps2.tile([128, NB], F32, tag="beta")
                        nc.tensor.matmul(bps[:], scT[:], mfi_sb[:],
                                         start=True, stop=True)
                        beta = asm.tile([128, NB], F32, tag="betas")
                        nc.vector.tensor_copy(beta[:], bps[:])
                        # res = sum_i beta_i * Bx_i
                        res = asb.tile([128, HID], BF16, tag="res")
                        tmp = asb.tile([128, HID], BF16, tag="tmp")
                        nc.vector.tensor_scalar_mul(res[:], zown_sb[:, 0, mch, :],
                                                    beta[:, 0:1])
                        for i in range(1, NB):
                            nc.vector.tensor_scalar_mul(
                                tmp[:], zown_sb[:, i, mch, :], beta[:, i:i + 1])
                            nc.vector.tensor_add(res[:], res[:], tmp[:])
                        for j in range(2):
                            tp2 = aps2.tile([128, 128], BF16, tag="rT")
                            nc.tensor.transpose(
                                tp2[:], res[:, j * 128:(j + 1) * 128], idb[:])
                            nc.scalar.copy(
                                resT_sb[:, j, mch * 128:(mch + 1) * 128], tp2[:])
                        # ship each res^T half as soon as its chunks exist --
                        # the AllGather overlaps the rest of the attention math
                        if phases >= 4 and mch in (4, MCH - 1):
                            h = 0 if mch == 4 else 1
                            nc.sync.dma_start(
                                rbounces[h][:],
                                resT_sb[:, :, h * 640:(h + 1) * 640])
                            nc.gpsimd.collective_compute(
                                "AllGather", ALU.bypass,
                                replica_groups=[list(range(NCORES))],
                                ins=[rbounces[h][:]], outs=[rgaths[h][:]])

            # =================== Phase 4: out = tanh(res @ res^T) ===========
            if phases >= 4:
                with tc.tile_pool(name="fin_ps", bufs=2, space="PSUM") as fps, \
                     tc.tile_pool(name="fin_rhs", bufs=2) as frhs, \
                     tc.tile_pool(name="fin_out", bufs=3) as fout:
                    # (noff, nw, h): psum column slice / which gathered half,
                    # each slice within one PSUM bank
                    nslf = [(0, 512, 0), (512, 128, 0), (640, 384, 1),
                            (1024, 226, 1)]
                    for blk in range(NCORES):
                        rt0 = frhs.tile([128, 2, 640], BF16, tag="rt0")
                        nc.sync.dma_start(rt0[:], rgaths[0][blk])
                        rt1 = frhs.tile([128, 2, 640], BF16, tag="rt1")
                        nc.sync.dma_start(rt1[:], rgaths[1][blk])
                        rts = (rt0, rt1)
                        for mch in range(MCH):
                            ps = fps.tile([128, SH], F32, tag="o")
                            for (noff, nw, h) in nslf:
                                for j in range(2):
                                    nc.tensor.matmul(
                                        ps[:, noff:noff + nw],
                                        resT_sb[:, j, mch * 128:(mch + 1) * 128],
                                        rts[h][:, j,
                                               noff - h * 640:noff - h * 640 + nw],
                                        start=(j == 0), stop=(j == 1))
                            ot = fout.tile([128, SH], BF16, tag="ot")
                            nc.scalar.activation(ot[:], ps[:], AF.Tanh)
                            rows = min(128, SH - mch * 128)
                            nc.sync.dma_start(
                                out_d[mch * 128:mch * 128 + rows,
                                      blk * SH:(blk + 1) * SH],
                                ot[:rows, :])
    nc.finalize()
    return nc


def _host_prep(x, edge_index, W1, b1, W2, b2, filt_w, Wf, bf, Wx, bx, lam):
    x = np.asarray(x, np.float32)
    ei = np.asarray(edge_index)
    src = ei[0].astype(np.int64)
    dst = ei[1].astype(np.int64)
    nonself = src != dst
    deg = np.bincount(src[nonself], minlength=N).astype(np.float32)
    dis = np.where(deg > 0,
                   1.0 / np.sqrt(np.maximum(deg, 1e-12)), 0.0).astype(np.float32)
    w = np.where(nonself, -(dis[src] * dis[dst]) / 2.0, 0.0).astype(np.float32)
    psrc = _pad_id(src)

    amats = []
    for c in range(NCORES):
        m = (dst >= c * SH) & (dst < (c + 1) * SH)
        at = np.zeros((NP_, SHP), np.float32)
        np.add.at(at, (psrc[m], dst[m] - c * SH), w[m])
        gids = np.arange(c * SH, (c + 1) * SH)
        at[_pad_id(gids), gids - c * SH] += 0.5
        amats.append((at * ASCALE).astype(NPF8))

    xtp = np.zeros((NP_, IN), np.float32)
    xtp[_pad_id(np.arange(N))] = x
    xt_all = np.ascontiguousarray(xtp.T).astype(NPBF16)
    xtowns = [np.ascontiguousarray(xtp[c * SHP:(c + 1) * SHP].T).astype(NPBF16)
              for c in range(NCORES)]

    W1 = np.asarray(W1, np.float32)
    W2 = np.asarray(W2, np.float32)
    Wf = np.asarray(Wf, np.float32)
    Wx = np.asarray(Wx, np.float32)
    b1 = np.asarray(b1, np.float32)
    b2 = np.asarray(b2, np.float32)
    bf = np.asarray(bf, np.float32)
    bx = np.asarray(bx, np.float32)
    filt_w = np.asarray(filt_w, np.float64)
    lam = np.asarray(lam, np.float64)

    C = _bern_coeff(KDEG)
    alpha = (1.0 / (1.0 + np.exp(-filt_w)) @ C).astype(np.float32)  # [F, NB]
    gate = np.concatenate([[1.0], 1.0 / (1.0 + np.exp(-lam[1:]))]
                          ).astype(np.float32)                       # [F]
    mfi = (gate[:, None] * alpha).astype(np.float32)                 # [F, NB]

    # W_big[i*256+r, f*256+c] = alpha[f, i] * Wf[r, c]
    wbig = (alpha.T[:, None, :, None] * Wf[None, :, None, :]).reshape(
        NB * HID, FNUM * HID)
    wbig = np.ascontiguousarray(
        wbig.reshape(2 * NB, 128, FNUM * HID).transpose(1, 0, 2) * WSCALE
        ).astype(NPF8)

    common = {
        "xt": xt_all,
        "w1": W1.astype(NPBF16),
        "w2": np.ascontiguousarray(
            W2.reshape(2, 128, HID).transpose(1, 0, 2)).astype(NPBF16),
        "wx": np.ascontiguousarray(
            Wx.reshape(2, 128, HID).transpose(1, 0, 2)).astype(NPBF16),
        "wbig": wbig,
        "b1c": np.ascontiguousarray(b1.reshape(2, 128).T).astype(np.float32),
        "b2r": b2.reshape(1, HID).astype(NPBF16),
        "bxr": bx.reshape(1, HID).astype(NPBF16),
        "bfc": (np.tile(bf, FNUM) * WSCALE).reshape(
            1, FNUM * HID).astype(NPBF16),
        "mfi": mfi,
    }
    in_maps = []
    for c in range(NCORES):
        m = dict(common)
        m["amat"] = amats[c]
        m["xtown"] = xtowns[c]
        in_maps.append(m)
    return in_maps


def _install_profile_shim():
    import sys, types
    if "antenv.axon_hooks" in sys.modules:
        return
    try:
        from trn_agent_boot.trn_boot import _ntff_profile_via_ctypes
        hook = _ntff_profile_via_ctypes("/opt/axon/libaxon_pjrt.so")
    except Exception:
        hook = None
    mod = types.ModuleType("antenv.axon_hooks")
    mod._hook = hook
    mod.get_axon_ntff_profile_hook = lambda: mod._hook
    mod.set_axon_ntff_profile_hook = lambda h: setattr(mod, "_hook", h)
    sys.modules["antenv.axon_hooks"] = mod
    try:
        import antenv
        antenv.axon_hooks = mod
    except Exception:
        pass


_NC_CACHE = None


def kernel(**inputs) -> np.ndarray:
    global _NC_CACHE
    t0 = time.time()
    in_maps = _host_prep(**inputs)
    t1 = time.time()
    skip_bias = all(
        float(np.abs(np.asarray(inputs[k])).max()) == 0.0
        for k in ("b1", "b2", "bf", "bx"))
    if _NC_CACHE is None:
        _NC_CACHE = build_nc(skip_bias=skip_bias)
    nc = _NC_CACHE
    t2 = time.time()
    trace = os.environ.get("KERNEL_TRACE", "") == "1"
    if trace:
        _install_profile_shim()
    res = run_bass_kernel_spmd(nc, in_maps, core_ids=list(range(NCORES)),
                               trace=trace)
    t3 = time.time()
    out = np.concatenate(
        [res.results[c]["out"].astype(np.float32) for c in range(NCORES)],
        axis=0)
    t4 = time.time()
    print(f"[kernel] host_prep={t1-t0:.2f}s build={t2-t1:.2f}s "
          f"run={t3-t2:.2f}s gather={t4-t3:.2f}s", flush=True)
    if trace and res.exec_time_ns is not None:
        print(f"HW exec time: {res.exec_time_ns} ns", flush=True)
        if res.instructions_and_trace:
            print(f"trace: {res.instructions_and_trace[1]}", flush=True)
    return out



# revision 17
# speedup vs baseline: 1.0855x; 1.0855x over previous
"""AMNet GNN forward pass on 8 Trainium2 NeuronCores (Bass/Tile SPMD).

Strategy:
  - Graph preprocessing on host: build the normalized-Laplacian propagation
    matrix A (dense bf16, diagonal 1/lambda_max folded in), sharded by
    destination rows across 8 cores, stored transposed ([src, dst_local])
    so it can be used directly as the matmul stationary operand.
  - Node dim padded 10000 -> 10240 (1280 per core) so everything tiles by 128.
  - MLP replicated on every core (cheap, avoids an AllGather of h).
  - 5 Bernstein propagations: z_{i+1} = A @ z_i as dense TensorE matmuls in
    fp8 e4m3 DoubleRow mode (A host-scaled by 32, descale fused into the
    psum casts; z kept bf16-accurate for attention via a separate staging
    path), z chunks stationary / A streaming, chunked AllGather of the fp8
    z shard (partition-major layout) overlapped with the other half's
    compute.
  - Attention: hfp = tanh(Bxcat @ W_big) with W_big = host-precomputed
    [alpha[f,i] * Wf] stack (folds the Bernstein combination into the GEMM),
    softmax over 5 filters, res = sum_i beta_i(n) * Bx_i.
  - Final: out = tanh(res @ res.T) as a row-sharded GEMM with res^T
    all-gathered; tanh on ScalarE; bf16 output, upcast to f32 on host.
"""

import os
import math
import time

import numpy as np
import ml_dtypes

import concourse.bass as bass
import concourse.mybir as mybir
import concourse.tile as tile
from concourse import bacc
from concourse.bass_utils import run_bass_kernel_spmd
from concourse.masks import make_identity

BF16 = mybir.dt.bfloat16
F8 = mybir.dt.float8e4
ASCALE = 32.0
WSCALE = 16.0
F32 = mybir.dt.float32
NPBF16 = ml_dtypes.bfloat16
NPF8 = ml_dtypes.float8_e4m3

N = 10000          # nodes
IN = 128           # input features
HID = 256          # hidden
FNUM = 5           # filters
KDEG = 5           # Bernstein degree
NB = KDEG + 1      # basis count
NCORES = 8
SH = 1250          # real nodes per core
SHP = 1280         # padded nodes per core
NP_ = NCORES * SHP # padded global nodes (10240)
MCH = SHP // 128   # m-chunks per core (10)
KCH = NP_ // 128   # k-chunks global (80)
SHARED_SPACE = os.environ.get("KERNEL_SHARED", "Shared")
AF = mybir.ActivationFunctionType
ALU = mybir.AluOpType


def _bern_coeff(deg):
    C = np.zeros((deg + 1, deg + 1), dtype=np.float64)
    for k in range(deg + 1):
        for i in range(k, deg + 1):
            C[k, i] = (
                math.comb(deg, k) * math.comb(deg - k, i - k) * ((-1.0) ** (i - k))
            )
    return C


def _pad_id(g):
    return (g // SH) * SHP + (g % SH)


def build_nc(phases=None, skip_bias=False):
    if phases is None:
        phases = int(os.environ.get("KERNEL_PHASES", "4") or "4")
    sub = os.environ.get("KERNEL_SUB", "z")
    nc = bacc.Bacc("TRN2", target_bir_lowering=False, debug=False,
                   num_devices=NCORES)

    amat = nc.declare_dram_parameter("amat", [NP_, SHP], F8, isOutput=False)
    xt = nc.declare_dram_parameter("xt", [IN, NP_], BF16, isOutput=False)
    xtown = nc.declare_dram_parameter("xtown", [IN, SHP], BF16, isOutput=False)
    w1_d = nc.declare_dram_parameter("w1", [IN, HID], BF16, isOutput=False)
    w2_d = nc.declare_dram_parameter("w2", [128, 2, HID], BF16, isOutput=False)
    wx_d = nc.declare_dram_parameter("wx", [128, 2, HID], BF16, isOutput=False)
    wbig_d = nc.declare_dram_parameter("wbig", [128, 12, FNUM * HID], F8,
                                       isOutput=False)
    b1_d = nc.declare_dram_parameter("b1c", [128, 2], F32, isOutput=False)
    b2_d = nc.declare_dram_parameter("b2r", [1, HID], BF16, isOutput=False)
    bx_d = nc.declare_dram_parameter("bxr", [1, HID], BF16, isOutput=False)
    bf_d = nc.declare_dram_parameter("bfc", [1, FNUM * HID], BF16, isOutput=False)
    mfi_d = nc.declare_dram_parameter("mfi", [FNUM, NB], F32, isOutput=False)
    out_d = nc.declare_dram_parameter("out", [SH, N], BF16, isOutput=True)

    with tile.TileContext(nc) as tc:
        with tc.tile_pool(name="const", bufs=1) as cpool, \
             tc.tile_pool(name="persist", bufs=1) as per, \
             tc.tile_pool(name="dram", bufs=1, space="DRAM") as dpool:
            # ---- constants into SBUF ----
            w1_sb = cpool.tile([IN, HID], BF16)
            nc.sync.dma_start(w1_sb[:], w1_d[:])
            w2_sb = cpool.tile([128, 2, HID], BF16)
            nc.sync.dma_start(w2_sb[:], w2_d[:])
            wx_sb = cpool.tile([128, 2, HID], BF16)
            nc.sync.dma_start(wx_sb[:], wx_d[:])
            wbig_sb = cpool.tile([128, 12, FNUM * HID], F8)
            nc.sync.dma_start(wbig_sb[:], wbig_d[:])
            b1_sb = cpool.tile([128, 2], F32)
            nc.sync.dma_start(b1_sb[:], b1_d[:])
            b2_sb = cpool.tile([1, HID], BF16)
            nc.sync.dma_start(b2_sb[:], b2_d[:])
            bx_sb = cpool.tile([1, HID], BF16)
            nc.sync.dma_start(bx_sb[:], bx_d[:])
            bf_sb = cpool.tile([1, FNUM * HID], BF16)
            nc.sync.dma_start(bf_sb[:], bf_d[:])
            mfi_sb = cpool.tile([FNUM, NB], F32)
            nc.sync.dma_start(mfi_sb[:], mfi_d[:])
            ones_sb = cpool.tile([1, 128], BF16)
            nc.vector.memset(ones_sb[:], 1.0)
            idb = cpool.tile([128, 128], BF16)
            make_identity(nc, idb[:])
            idf = cpool.tile([128, 128], F32)
            make_identity(nc, idf[:])

            # ---- persistent SBUF state ----
            z_sb = per.tile([128, NCORES, MCH, HID], F8)      # full z (fp8)
            zof_sb = per.tile([128, MCH, HID], F8)            # own z fp8 stage
            zown_sb = per.tile([128, NB, MCH, HID], BF16)     # own Bx_i shards
            bxT_sb = per.tile([128, 2 * NB, SHP], BF16)       # own Bx_i^T
            bxT8_sb = per.tile([128, 2 * NB, SHP], F8)        # fp8 copy
            resT8_sb = per.tile([128, 2, SHP], F8)            # own res^T (fp8)
            xp_sb = per.tile([128, MCH, HID], BF16)           # tanh(h@Wx+bx)

            # ---- DRAM bounce buffers for collectives ----
            zbounces = [
                [dpool.tile([128, (6, 4)[g], HID], F8, name=f"zbounce{i}_{g}")
                 for g in range(2)]
                for i in range(KDEG - 1)
            ]
            zgaths = [
                [dpool.tile([NCORES, 128, (6, 4)[g], HID], F8,
                            addr_space=SHARED_SPACE, name=f"zgath{i}_{g}")
                 for g in range(2)]
                for i in range(KDEG - 1)
            ]
            rbounces = [dpool.tile([128, 2, 640], F8, name=f"rbounce{h}")
                        for h in range(2)]
            rgaths = [dpool.tile([NCORES, 128, 2, 640], F8,
                                 addr_space=SHARED_SPACE, name=f"rgath{h}")
                      for h in range(2)]

            # tiny warmup collective: pays the cold-start cost of the
            # collectives firmware under the MLP instead of in prop 0
            wub = dpool.tile([1, HID], BF16, name="wub")
            wug = dpool.tile([NCORES, HID], BF16, addr_space=SHARED_SPACE,
                             name="wug")
            nc.vector.memset(ones_sb[:], 1.0)
            nc.sync.dma_start(wub[:], b2_d[:])
            nc.gpsimd.collective_compute(
                "AllGather", ALU.bypass,
                replica_groups=[list(range(NCORES))],
                ins=[wub[:]], outs=[wug[:]])

            def bias_mm(ps_ap, brow_ap):
                # rank-1 matmul adding a free-dim bias row to every partition;
                # elided when the host saw all-zero biases
                if skip_bias:
                    return
                nc.tensor.matmul(ps_ap, ones_sb[0:1, :], brow_ap,
                                 start=False, stop=True)

            # =================== Phase 1: MLP ===================
            # t^T = relu(W1^T x^T + b1): full graph, replicated
            with tc.tile_pool(name="mlp_sb", bufs=1) as mpool, \
                 tc.tile_pool(name="mlp_xt", bufs=3) as xpool, \
                 tc.tile_pool(name="mlp_ps", bufs=4, space="PSUM") as mps:
                tT_sb = mpool.tile([128, 2, NP_], BF16)
                tTo_sb = mpool.tile([128, 2, SHP], BF16)
                for nt in range(NP_ // 512 if sub >= "a" else 0):
                    xt_t = xpool.tile([IN, 512], BF16, tag="xt")
                    nc.sync.dma_start(xt_t[:], xt[:, nt * 512:(nt + 1) * 512])
                    for m in range(2):
                        ps = mps.tile([128, 512], F32, tag="t", bufs=3)
                        nc.tensor.matmul(ps[:], w1_sb[:, m * 128:(m + 1) * 128],
                                         xt_t[:], start=True, stop=True)
                        if (nt + m) % 2 == 0:
                            nc.scalar.activation(
                                tT_sb[:, m, nt * 512:(nt + 1) * 512], ps[:],
                                AF.Relu, bias=b1_sb[:, m:m + 1])
                        else:
                            nc.vector.tensor_scalar(
                                out=tT_sb[:, m, nt * 512:(nt + 1) * 512],
                                in0=ps[:], scalar1=b1_sb[:, m:m + 1],
                                scalar2=0.0, op0=ALU.add, op1=ALU.max)
                # own-shard t^T (same math on own columns; gives rank's shard
                # without rank-dependent addressing)
                off = 0
                for w in ((512, 512, 256) if sub >= "b" else ()):
                    xt_t = xpool.tile([IN, 512], BF16, tag="xt")
                    nc.sync.dma_start(xt_t[:, :w], xtown[:, off:off + w])
                    for m in range(2):
                        ps = mps.tile([128, 512], F32, tag="t", bufs=3)
                        nc.tensor.matmul(ps[:, :w],
                                         w1_sb[:, m * 128:(m + 1) * 128],
                                         xt_t[:, :w], start=True, stop=True)
                        nc.scalar.activation(
                            tTo_sb[:, m, off:off + w], ps[:, :w],
                            AF.Relu, bias=b1_sb[:, m:m + 1])
                    off += w

                # h = t @ W2 + b2 (node-major), full graph -> z_sb
                for nch in range(KCH if sub >= "c" else 0):
                    ps = mps.tile([128, HID], F32, tag="h", bufs=3)
                    for j in range(2):
                        nc.tensor.matmul(
                            ps[:], tT_sb[:, j, nch * 128:(nch + 1) * 128],
                            w2_sb[:, j, :], start=(j == 0), stop=False)
                    bias_mm(ps[:], b2_sb[0:1, :])
                    if nch % 2 == 0:
                        nc.vector.tensor_copy(z_sb[:, nch // MCH, nch % MCH, :],
                                              ps[:])
                    else:
                        nc.scalar.copy(z_sb[:, nch // MCH, nch % MCH, :], ps[:])
                # own shard h -> zown[0] (+ transposes -> bxT[0])
                for mch in range(MCH if sub >= "d" else 0):
                    ps = mps.tile([128, HID], F32, tag="h", bufs=3)
                    for j in range(2):
                        nc.tensor.matmul(
                            ps[:], tTo_sb[:, j, mch * 128:(mch + 1) * 128],
                            w2_sb[:, j, :], start=(j == 0), stop=False)
                    bias_mm(ps[:], b2_sb[0:1, :])
                    nc.vector.tensor_copy(zown_sb[:, 0, mch, :], ps[:])
                    for j in range(2):
                        tp = mps.tile([128, 128], BF16, tag="tp", bufs=2)
                        nc.tensor.transpose(
                            tp[:], zown_sb[:, 0, mch, j * 128:(j + 1) * 128],
                            idb[:])
                        nc.scalar.copy(
                            bxT_sb[:, j, mch * 128:(mch + 1) * 128], tp[:])
                        nc.vector.tensor_copy(
                            bxT8_sb[:, j, mch * 128:(mch + 1) * 128], tp[:])

            if phases < 4:
                dummy = cpool.tile([128, 512], BF16, name="dummy")
                nc.vector.memset(dummy[:], 0.0)
                nc.sync.dma_start(out_d[0:128, 0:512], dummy[:])

            # =================== Phase 2: propagations ===================
            if phases >= 2:
                with tc.tile_pool(name="prop_ps", bufs=6, space="PSUM") as pps, \
                     tc.tile_pool(name="tp_ps", bufs=2, space="PSUM") as tps, \
                     tc.tile_pool(name="aslab", bufs=12) as apool:
                    # xp = tanh(h @ Wx + bx) -- depends only on bxT[0]; emitted
                    # early so it fills PE/ACT idle time during AG waits.
                    for mch in range(MCH):
                        ps = pps.tile([128, HID], F32, tag="acc")
                        for j in range(2):
                            nc.tensor.matmul(
                                ps[:], bxT_sb[:, j, mch * 128:(mch + 1) * 128],
                                wx_sb[:, j, :], start=(j == 0), stop=False)
                        bias_mm(ps[:], bx_sb[0:1, :])
                        nc.scalar.activation(xp_sb[:, mch, :], ps[:], AF.Tanh)

                    # Operand-swapped props: z chunks are the STATIONARY
                    # operand (one LDWEIGHTS per (k, hid-half), reused over
                    # m-tiles), A columns stream as the moving operand.  The
                    # psum output arrives hid-major and IS BxT; the
                    # node-major copy comes from the 20 PE transposes we paid
                    # anyway.  Two passes per prop over own node cols
                    # [0:512] (chunks 0-3) and [512:1280] (chunks 4-9); each
                    # pass's AllGather half ships while the other computes.
                    # k order: chunks 0-3 of every rank first, then 4-9, so
                    # the previous prop's first gathered half unblocks early.
                    korder = [r * (MCH // 2) + pc
                              for half in ((2, 3, 4), (0, 1))
                              for pc in half for r in range(NCORES)]
                    NPAIR = KCH // 2
                    PASSES = [(512, (512, 256), 4, MCH), (0, (512,), 0, 4)]
                    for i in range(KDEG):
                        war_guard = None
                        for g, (base, tiles, clo, chi) in enumerate(PASSES):
                            wcols = sum(tiles)
                            accs = {}
                            for j in range(2):
                                for ti in range(len(tiles)):
                                    accs[j, ti] = pps.tile(
                                        [128, 512], F32, tag="acc",
                                        name=f"acc_{i}_{g}_{j}_{ti}")
                            for ki, kp in enumerate(korder):
                                r, pc = kp // (MCH // 2), kp % (MCH // 2)
                                sl = apool.tile([128, 2, 768], F8, tag="a")
                                nc.sync.dma_start(
                                    sl[:, :, :wcols],
                                    amat[kp * 256:(kp + 1) * 256,
                                         base:base + wcols].rearrange(
                                             "(i p) m -> p i m", p=128))
                                for j in range(2):
                                    lhs = z_sb[:, r, 2 * pc:2 * pc + 2,
                                               j * 128:(j + 1) * 128]
                                    off = 0
                                    for ti, tw in enumerate(tiles):
                                        mm = nc.tensor.matmul(
                                            accs[j, ti][:, :tw], lhs,
                                            sl[:, :, off:off + tw],
                                            start=(ki == 0),
                                            stop=(ki == NPAIR - 1),
                                            perf_mode=(
                                                mybir.MatmulPerfMode.DoubleRow))
                                        off += tw
                                        if (g == 1 and ki == 3 * NCORES - 1
                                                and j == 1
                                                and ti == len(tiles) - 1):
                                            # last matmul of prop i reading
                                            # z_sb chunks 4-9 (PE in-order)
                                            war_guard = mm
                            # psums are z_{i+1}^T pieces -> straight to bxT
                            for j in range(2):
                                off = 0
                                for ti, tw in enumerate(tiles):
                                    dst = bxT_sb[:, (i + 1) * 2 + j,
                                                 base + off:base + off + tw]
                                    dst8 = bxT8_sb[:, (i + 1) * 2 + j,
                                                   base + off:base + off + tw]
                                    if (j + ti) % 2 == 0:
                                        nc.vector.tensor_scalar_mul(
                                            dst, accs[j, ti][:, :tw],
                                            1.0 / ASCALE)
                                        nc.scalar.activation(
                                            dst8, accs[j, ti][:, :tw], AF.Copy,
                                            scale=1.0 / ASCALE)
                                    else:
                                        nc.scalar.activation(
                                            dst, accs[j, ti][:, :tw], AF.Copy,
                                            scale=1.0 / ASCALE)
                                        nc.vector.tensor_scalar_mul(
                                            dst8, accs[j, ti][:, :tw],
                                            1.0 / ASCALE)
                                    off += tw
                            # node-major zown via PE transpose of bxT blocks
                            for mch in range(clo, chi):
                                for j in range(2):
                                    tp = tps.tile([128, 128], BF16, tag="tp")
                                    nc.tensor.transpose(
                                        tp[:],
                                        bxT_sb[:, (i + 1) * 2 + j,
                                               mch * 128:(mch + 1) * 128],
                                        idb[:])
                                    nc.scalar.copy(
                                        zown_sb[:, i + 1, mch,
                                                j * 128:(j + 1) * 128], tp[:])
                                    if i < KDEG - 1:
                                        nc.vector.tensor_copy(
                                            zof_sb[:, mch,
                                                   j * 128:(j + 1) * 128],
                                            tp[:])
                            # chunked AllGather of this pass's node chunks
                            if i < KDEG - 1:
                                nc.sync.dma_start(
                                    zbounces[i][g][:],
                                    zof_sb[:, clo:chi, :])
                                nc.gpsimd.collective_compute(
                                    "AllGather", ALU.bypass,
                                    replica_groups=[list(range(NCORES))],
                                    ins=[zbounces[i][g][:]],
                                    outs=[zgaths[i][g][:]])
                        # land the gathered halves into z_sb for prop i+1.
                        # The chunks 0-3 write-after-read hazard against this
                        # prop's late reads is not auto-tracked; pin it.
                        if i < KDEG - 1:
                            for g, (_, _, clo, chi) in enumerate(PASSES):
                                for r in range(NCORES):
                                    d = nc.sync.dma_start(
                                        z_sb[:, r, clo:chi, :],
                                        zgaths[i][g][r])
                                    if g == 0 and war_guard is not None:
                                        tile.add_dep_helper(
                                            d.ins, war_guard.ins, sync=True,
                                            reason="z_sb WAR vs prop reads")

            # =================== Phase 3: attention ===================
            if phases >= 3:
                with tc.tile_pool(name="att_ps", bufs=4, space="PSUM") as aps, \
                     tc.tile_pool(name="att_ps2", bufs=1, space="PSUM") as aps2, \
                     tc.tile_pool(name="att_sb", bufs=2) as asb, \
                     tc.tile_pool(name="att_small", bufs=4) as asm:
                    nsl = [(0, 512), (512, 512), (1024, FNUM * HID - 1024)]
                    for mch in range(MCH):
                        hfp = asb.tile([128, FNUM * HID], BF16, tag="hfp")
                        for (noff, nw) in nsl:
                            ps = aps.tile([128, 512], F32, tag="hf")
                            for pc in range(NB):
                                nc.tensor.matmul(
                                    ps[:, :nw],
                                    bxT8_sb[:, 2 * pc:2 * pc + 2,
                                            mch * 128:(mch + 1) * 128],
                                    wbig_sb[:, 2 * pc:2 * pc + 2,
                                            noff:noff + nw],
                                    start=(pc == 0),
                                    stop=(pc == NB - 1 and skip_bias),
                                    perf_mode=mybir.MatmulPerfMode.DoubleRow)
                            nc.tensor.matmul(ps[:, :nw], ones_sb[0:1, :],
                                             bf_sb[0:1, noff:noff + nw],
                                             start=False, stop=True) \
                                if not skip_bias else None
                            nc.scalar.activation(hfp[:, noff:noff + nw],
                                                 ps[:, :nw], AF.Tanh,
                                                 scale=1.0 / WSCALE)
                        # logits over filters
                        logit = asm.tile([128, FNUM], F32, tag="logit")
                        scr = asb.tile([128, HID], BF16, tag="scr")
                        for f in range(FNUM):
                            nc.vector.tensor_mul(
                                scr[:], hfp[:, f * HID:(f + 1) * HID],
                                xp_sb[:, mch, :])
                            nc.vector.reduce_sum(
                                out=logit[:, f:f + 1], in_=scr[:],
                                axis=mybir.AxisListType.X)
                        # softmax over the FNUM free dim
                        mx = asm.tile([128, 1], F32, tag="mx")
                        nc.vector.reduce_max(out=mx[:], in_=logit[:],
                                             axis=mybir.AxisListType.X)
                        score = asm.tile([128, FNUM], F32, tag="score")
                        nc.vector.tensor_scalar(out=score[:], in0=logit[:],
                                                scalar1=mx[:, 0:1], scalar2=None,
                                                op0=ALU.subtract)
                        nc.scalar.activation(score[:], score[:], AF.Exp)
                        sm = asm.tile([128, 1], F32, tag="sm")
                        nc.vector.reduce_sum(out=sm[:], in_=score[:],
                                             axis=mybir.AxisListType.X)
                        rs = asm.tile([128, 1], F32, tag="rs")
                        nc.vector.reciprocal(rs[:], sm[:])
                        nc.vector.tensor_scalar_mul(score[:], score[:], rs[:, 0:1])
                        # beta = score @ (gate*alpha):  transpose score, small GEMM
                        tp = aps2.tile([FNUM, 128], F32, tag="scT")
                        nc.tensor.transpose(tp[:], score[:], idf[:])
                        scT = asm.tile([FNUM, 128], F32, tag="scTs")
                        nc.vector.tensor_copy(scT[:], tp[:])
                        bps = aps2.tile([128, NB], F32, tag="beta")
                        nc.tensor.matmul(bps[:], scT[:], mfi_sb[:],
                                         start=True, stop=True)
                        beta = asm.tile([128, NB], F32, tag="betas")
                        nc.vector.tensor_copy(beta[:], bps[:])
                        # res = sum_i beta_i * Bx_i
                        res = asb.tile([128, HID], BF16, tag="res")
                        tmp = asb.tile([128, HID], BF16, tag="tmp")
                        nc.vector.tensor_scalar_mul(res[:], zown_sb[:, 0, mch, :],
                                                    beta[:, 0:1])
                        for i in range(1, NB):
                            nc.vector.tensor_scalar_mul(
                                tmp[:], zown_sb[:, i, mch, :], beta[:, i:i + 1])
                            nc.vector.tensor_add(res[:], res[:], tmp[:])
                        for j in range(2):
                            tp2 = aps2.tile([128, 128], BF16, tag="rT")
                            nc.tensor.transpose(
                                tp2[:], res[:, j * 128:(j + 1) * 128], idb[:])
                            if j == 0:
                                nc.scalar.copy(
                                    resT8_sb[:, j, mch * 128:(mch + 1) * 128],
                                    tp2[:])
                            else:
                                nc.vector.tensor_copy(
                                    resT8_sb[:, j, mch * 128:(mch + 1) * 128],
                                    tp2[:])
                        # ship each res^T half as soon as its chunks exist --
                        # the AllGather overlaps the rest of the attention math
                        if phases >= 4 and mch in (4, MCH - 1):
                            h = 0 if mch == 4 else 1
                            nc.sync.dma_start(
                                rbounces[h][:],
                                resT8_sb[:, :, h * 640:(h + 1) * 640])
                            nc.gpsimd.collective_compute(
                                "AllGather", ALU.bypass,
                                replica_groups=[list(range(NCORES))],
                                ins=[rbounces[h][:]], outs=[rgaths[h][:]])

            # =================== Phase 4: out = tanh(res @ res^T) ===========
            if phases >= 4:
                with tc.tile_pool(name="fin_ps", bufs=2, space="PSUM") as fps, \
                     tc.tile_pool(name="fin_rhs", bufs=2) as frhs, \
                     tc.tile_pool(name="fin_out", bufs=3) as fout:
                    # (noff, nw, h): psum column slice / which gathered half,
                    # each slice within one PSUM bank
                    nslf = [(0, 512, 0), (512, 128, 0), (640, 384, 1),
                            (1024, 226, 1)]
                    for blk in range(NCORES):
                        rt0 = frhs.tile([128, 2, 640], F8, tag="rt0")
                        nc.sync.dma_start(rt0[:], rgaths[0][blk])
                        rt1 = frhs.tile([128, 2, 640], F8, tag="rt1")
                        nc.sync.dma_start(rt1[:], rgaths[1][blk])
                        rts = (rt0, rt1)
                        for mch in range(MCH):
                            ps = fps.tile([128, SH], F32, tag="o")
                            for (noff, nw, h) in nslf:
                                nc.tensor.matmul(
                                    ps[:, noff:noff + nw],
                                    resT8_sb[:, :, mch * 128:(mch + 1) * 128],
                                    rts[h][:, :,
                                           noff - h * 640:noff - h * 640 + nw],
                                    start=True, stop=True,
                                    perf_mode=mybir.MatmulPerfMode.DoubleRow)
                            ot = fout.tile([128, SH], BF16, tag="ot")
                            nc.scalar.activation(ot[:], ps[:], AF.Tanh)
                            rows = min(128, SH - mch * 128)
                            nc.sync.dma_start(
                                out_d[mch * 128:mch * 128 + rows,
                                      blk * SH:(blk + 1) * SH],
                                ot[:rows, :])
    nc.finalize()
    return nc


def _host_prep(x, edge_index, W1, b1, W2, b2, filt_w, Wf, bf, Wx, bx, lam):
    x = np.asarray(x, np.float32)
    ei = np.asarray(edge_index)
    src = ei[0].astype(np.int64)
    dst = ei[1].astype(np.int64)
    nonself = src != dst
    deg = np.bincount(src[nonself], minlength=N).astype(np.float32)
    dis = np.where(deg > 0,
                   1.0 / np.sqrt(np.maximum(deg, 1e-12)), 0.0).astype(np.float32)
    w = np.where(nonself, -(dis[src] * dis[dst]) / 2.0, 0.0).astype(np.float32)
    psrc = _pad_id(src)

    amats = []
    for c in range(NCORES):
        m = (dst >= c * SH) & (dst < (c + 1) * SH)
        at = np.zeros((NP_, SHP), np.float32)
        np.add.at(at, (psrc[m], dst[m] - c * SH), w[m])
        gids = np.arange(c * SH, (c + 1) * SH)
        at[_pad_id(gids), gids - c * SH] += 0.5
        amats.append((at * ASCALE).astype(NPF8))

    xtp = np.zeros((NP_, IN), np.float32)
    xtp[_pad_id(np.arange(N))] = x
    xt_all = np.ascontiguousarray(xtp.T).astype(NPBF16)
    xtowns = [np.ascontiguousarray(xtp[c * SHP:(c + 1) * SHP].T).astype(NPBF16)
              for c in range(NCORES)]

    W1 = np.asarray(W1, np.float32)
    W2 = np.asarray(W2, np.float32)
    Wf = np.asarray(Wf, np.float32)
    Wx = np.asarray(Wx, np.float32)
    b1 = np.asarray(b1, np.float32)
    b2 = np.asarray(b2, np.float32)
    bf = np.asarray(bf, np.float32)
    bx = np.asarray(bx, np.float32)
    filt_w = np.asarray(filt_w, np.float64)
    lam = np.asarray(lam, np.float64)

    C = _bern_coeff(KDEG)
    alpha = (1.0 / (1.0 + np.exp(-filt_w)) @ C).astype(np.float32)  # [F, NB]
    gate = np.concatenate([[1.0], 1.0 / (1.0 + np.exp(-lam[1:]))]
                          ).astype(np.float32)                       # [F]
    mfi = (gate[:, None] * alpha).astype(np.float32)                 # [F, NB]

    # W_big[i*256+r, f*256+c] = alpha[f, i] * Wf[r, c]
    wbig = (alpha.T[:, None, :, None] * Wf[None, :, None, :]).reshape(
        NB * HID, FNUM * HID)
    wbig = np.ascontiguousarray(
        wbig.reshape(2 * NB, 128, FNUM * HID).transpose(1, 0, 2) * WSCALE
        ).astype(NPF8)

    common = {
        "xt": xt_all,
        "w1": W1.astype(NPBF16),
        "w2": np.ascontiguousarray(
            W2.reshape(2, 128, HID).transpose(1, 0, 2)).astype(NPBF16),
        "wx": np.ascontiguousarray(
            Wx.reshape(2, 128, HID).transpose(1, 0, 2)).astype(NPBF16),
        "wbig": wbig,
        "b1c": np.ascontiguousarray(b1.reshape(2, 128).T).astype(np.float32),
        "b2r": b2.reshape(1, HID).astype(NPBF16),
        "bxr": bx.reshape(1, HID).astype(NPBF16),
        "bfc": (np.tile(bf, FNUM) * WSCALE).reshape(
            1, FNUM * HID).astype(NPBF16),
        "mfi": mfi,
    }
    in_maps = []
    for c in range(NCORES):
        m = dict(common)
        m["amat"] = amats[c]
        m["xtown"] = xtowns[c]
        in_maps.append(m)
    return in_maps


def _install_profile_shim():
    import sys, types
    if "antenv.axon_hooks" in sys.modules:
        return
    try:
        from trn_agent_boot.trn_boot import _ntff_profile_via_ctypes
        hook = _ntff_profile_via_ctypes("/opt/axon/libaxon_pjrt.so")
    except Exception:
        hook = None
    mod = types.ModuleType("antenv.axon_hooks")
    mod._hook = hook
    mod.get_axon_ntff_profile_hook = lambda: mod._hook
    mod.set_axon_ntff_profile_hook = lambda h: setattr(mod, "_hook", h)
    sys.modules["antenv.axon_hooks"] = mod
    try:
        import antenv
        antenv.axon_hooks = mod
    except Exception:
        pass


_NC_CACHE = None


def kernel(**inputs) -> np.ndarray:
    global _NC_CACHE
    t0 = time.time()
    in_maps = _host_prep(**inputs)
    t1 = time.time()
    skip_bias = all(
        float(np.abs(np.asarray(inputs[k])).max()) == 0.0
        for k in ("b1", "b2", "bf", "bx"))
    if _NC_CACHE is None:
        _NC_CACHE = build_nc(skip_bias=skip_bias)
    nc = _NC_CACHE
    t2 = time.time()
    trace = os.environ.get("KERNEL_TRACE", "") == "1"
    if trace:
        _install_profile_shim()
    res = run_bass_kernel_spmd(nc, in_maps, core_ids=list(range(NCORES)),
                               trace=trace)
    t3 = time.time()
    out = np.concatenate(
        [res.results[c]["out"].astype(np.float32) for c in range(NCORES)],
        axis=0)
    t4 = time.time()
    print(f"[kernel] host_prep={t1-t0:.2f}s build={t2-t1:.2f}s "
          f"run={t3-t2:.2f}s gather={t4-t3:.2f}s", flush=True)
    if trace and res.exec_time_ns is not None:
        print(f"HW exec time: {res.exec_time_ns} ns", flush=True)
        if res.instructions_and_trace:
            print(f"trace: {res.instructions_and_trace[1]}", flush=True)
    return out

